# revision 34
# baseline (speedup 1.0000x reference)
"""Trainium2 Bass kernel for nn_BlockDiagonal.

Math: out = x @ tanh(W * mask).T, where mask is block-diagonal with 64
blocks of 64x64. tanh(0) = 0, so the effective weight is block-diagonal:
out[:, 64j:64j+64] = x[:, 64j:64j+64] @ tanh(Wb_j).T for each block j —
64 independent small GEMMs, 1/64th of the dense FLOPs.

Strategy (8 NeuronCores, data parallel over rows; ~32MB of HBM traffic
per core makes this DMA/PE balanced — "ridge"):
- Each core owns 1024 rows of x / out.
- Host (numpy, data movement only): verify the mask is block-diagonal,
  gather the 64 diagonal 64x64 blocks of W (mask applied via np.where
  selection), pack pairs of blocks into [128,128] block-diagonal tiles,
  transposed, zeros off-block -> wt [128, 4096]. All arithmetic (tanh,
  matmul) runs on device; tanh maps the structural zeros to zero.
- Device per core, all fp32 (exact to ~1e-7):
  - weights: 4 [128,1024] quarter tiles DMA'd on the Scalar HWDGE ring
    (idle until stores), tanh'd in place on ACT.
  - x streams on the Sync HWDGE ring: chunk 0 in 512KB quarters (fast
    pipeline start), later chunks in 1MB halves. Separate rings for
    loads vs stores avoid FIFO head-of-line blocking.
  - per [128 rows, 1024 cols] batch: 8 PE transposes (identity built
    on-device) -> PSUM -> DVE copy to SBUF (xt), then 8 fp32 matmuls
    lhsT=xt[feat,rows], rhs=tanh-tile [feat_in,feat_out] -> PSUM ->
    ACT copy -> SBUF -> store. Matmul batches run one stage behind
    transpose batches so PE never waits on the DVE copy.
"""

import os
import sys

import numpy as np

for _p in ("/opt/trn_rl_repo", "/root/.axon_site/_ro/trn_rl_repo"):
    if os.path.isdir(_p) and _p not in sys.path:
        sys.path.append(_p)

from contextlib import ExitStack

# If BASS_TRACE is set in the environment, bass_utils' axon path imports
# antenv.axon_hooks, which this image lacks — stub it (hook=None means
# "run without tracing") so kernel() can never crash on it.
try:
    from antenv.axon_hooks import get_axon_ntff_profile_hook  # noqa: F401
except ImportError:
    import types

    import antenv

    _stub = types.ModuleType("antenv.axon_hooks")
    _stub._hook = None
    _stub.get_axon_ntff_profile_hook = lambda: _stub._hook

    def _set_hook(h):
        _stub._hook = h

    _stub.set_axon_ntff_profile_hook = _set_hook
    sys.modules["antenv.axon_hooks"] = _stub
    antenv.axon_hooks = _stub

import concourse.bacc as bacc
from concourse import mybir
from concourse.bass_utils import run_bass_kernel_spmd
from concourse.masks import make_identity
from concourse.tile import TileContext

ROWS = 8192
L = 4096
NB = 64           # number of diagonal blocks
B = 64            # block size
NCORES = 8
R = ROWS // NCORES  # rows per core = 1024
P = 128             # partition tile
NG = L // P         # 32 column groups (2 blocks each)
NCHUNK = R // P     # 8 row chunks per core

F32 = mybir.dt.float32

_NC_CACHE = {}


def build_nc():
    """Build the per-core Bass program (SPMD: same program, different data)."""
    nc = bacc.Bacc()
    x_in = nc.declare_dram_parameter("x", [R, L], F32, isOutput=False)
    wt_in = nc.declare_dram_parameter("wt", [P, L], F32, isOutput=False)
    out_ext = nc.declare_dram_parameter("out", [R, L], F32, isOutput=True)

    with TileContext(nc) as tc, ExitStack() as ctx:
        singles = ctx.enter_context(tc.tile_pool(name="singles", bufs=1))
        wpool = ctx.enter_context(tc.tile_pool(name="wpool", bufs=4))
        xpool = ctx.enter_context(tc.tile_pool(name="xpool", bufs=4))
        xtpool = ctx.enter_context(tc.tile_pool(name="xtpool", bufs=6))
        opool = ctx.enter_context(tc.tile_pool(name="opool", bufs=4))
        pt = ctx.enter_context(tc.tile_pool(name="pt", bufs=2, space="PSUM"))
        pm = ctx.enter_context(tc.tile_pool(name="pm", bufs=2, space="PSUM"))

        Q = 1024        # quarter-chunk columns = one batch of 8 column groups
        H = L // 2      # half-chunk columns (store granularity)

        ident = singles.tile([P, P], F32)
        make_identity(nc, ident)


        # x is loaded in quarter-chunks, one tile per (chunk, q) batch, so the
        # first transposes start as soon as 512KB has landed. Loads ride the
        # Sync HWDGE ring. Weights ride the Scalar ring in quarters too:
        # batch q's matmuls need exactly weight quarter q.
        def load_xq(ic, q):
            xq = xpool.tile([P, Q], F32, tag="xs", name=f"xq_{ic}_{q}", bufs=8)
            nc.sync.dma_start(
                out=xq, in_=x_in[ic * P:(ic + 1) * P, Q * q:Q * (q + 1)],
            )
            return xq

        first_q = {ic: [load_xq(ic, q) for q in range(4)] for ic in range(2)}

        wts = []
        for w in range(4):
            wq = wpool.tile([P, Q], F32, tag="wt", name=f"wt_{w}")
            # quarters 0-1 are needed almost immediately: HWDGE scalar ring.
            # quarters 2-3 are needed ~8us later: the idle GpSimd SWDGE queue
            # handles them without stealing early HWDGE bandwidth from x.
            eng = nc.scalar if w < 2 else nc.gpsimd
            eng.dma_start(out=wq, in_=wt_in[:, Q * w:Q * (w + 1)])
            wts.append(wq)
        for w in range(4):
            for s in range(2):
                nc.scalar.activation(
                    out=wts[w][:, 512 * s:512 * (s + 1)],
                    in_=wts[w][:, 512 * s:512 * (s + 1)],
                    func=mybir.ActivationFunctionType.Tanh,
                )

        def mm_batch(ic, q, xt, last, ohalves):
            po = pm.tile([P, Q], F32, tag="po", name=f"po_{ic}_{q}")
            for t in range(8):
                nc.tensor.matmul(
                    po[:, P * t:P * (t + 1)],
                    lhsT=xt[:, P * t:P * (t + 1)],
                    rhs=wts[q][:, P * t:P * (t + 1)],
                    start=True,
                    stop=True,
                )
            # PSUM->SBUF out copies on ACT (DVE stays dedicated to the xt
            # copies that feed PE); last chunk splits to DVE since no more
            # xt copies are coming.
            oh = ohalves[q // 2]
            dst = oh[:, Q * (q % 2):Q * (q % 2 + 1)]
            if last:
                # drain tail fast: halves copied on DVE and ACT in parallel
                nc.vector.tensor_copy(out=dst[:, 0:512], in_=po[:, 0:512])
                nc.scalar.copy(out=dst[:, 512:1024], in_=po[:, 512:1024])
            else:
                nc.scalar.copy(out=dst, in_=po)
            # stores ride the Scalar HWDGE ring; per half chunk, but
            # quarter-chunk on the last chunk to shorten the drain tail
            if last:
                nc.scalar.dma_start(
                    out=out_ext[ic * P:(ic + 1) * P, Q * q:Q * (q + 1)],
                    in_=dst,
                )
            elif q % 2 == 1:
                h = q // 2
                nc.scalar.dma_start(
                    out=out_ext[ic * P:(ic + 1) * P, h * H:(h + 1) * H],
                    in_=oh,
                )

        def load_xh(ic, h):
            xh = xpool.tile([P, H], F32, tag="xs2", name=f"xh_{ic}_{h}", bufs=6)
            nc.sync.dma_start(
                out=xh, in_=x_in[ic * P:(ic + 1) * P, h * H:(h + 1) * H],
            )
            return xh

        pending = None  # one-stage pipeline skew on PE
        for ic in range(NCHUNK):
            last = ic == NCHUNK - 1
            halves = None if ic < 2 else [load_xh(ic, h) for h in range(2)]
            ohalves = [
                opool.tile([P, H], F32, tag="outs", name=f"oh_{ic}_{h}")
                for h in range(2)
            ]
            for q in range(4):
                if ic < 2:
                    xq, off = first_q[ic][q], 0
                else:
                    xq, off = halves[q // 2], Q * (q % 2)
                tp = pt.tile([P, Q], F32, tag="tp", name=f"tp_{ic}_{q}")
                for t in range(8):
                    nc.tensor.transpose(
                        tp[:, P * t:P * (t + 1)],
                        xq[:, off + P * t:off + P * (t + 1)],
                        ident,
                    )
                xt = xtpool.tile([P, Q], F32, tag="xt", name=f"xt_{ic}_{q}")
                nc.vector.tensor_copy(out=xt, in_=tp)
                if ic == 0 and q < 2:
                    # pipeline head: no skew — x0q1 hasn't landed yet, so
                    # emitting mm(0,q) immediately gives PE ready work
                    # instead of stalling on the next transpose batch.
                    mm_batch(ic, q, xt, last, ohalves)
                else:
                    if pending is not None:
                        mm_batch(*pending)
                    pending = (ic, q, xt, last, ohalves)
        mm_batch(*pending)

    nc.compile()
    return nc


def _get_nc():
    if "nc" not in _NC_CACHE:
        _NC_CACHE["nc"] = build_nc()
    return _NC_CACHE["nc"]


def _mask_is_block_diagonal(mask: np.ndarray) -> bool:
    off = mask.copy()
    for i in range(NB):
        s = i * B
        off[s:s + B, s:s + B] = False
    return not off.any()


def _build_wt(W: np.ndarray, mask: np.ndarray) -> np.ndarray:
    """Pack the 64 diagonal blocks, transposed, into [128, 4096]:
    wt[r, 128*g + c] = (W*mask).T within block pair g, zeros off-block."""
    Wm = np.where(mask, W, np.float32(0.0)).astype(np.float32)
    wt_g = np.zeros((NG, P, P), dtype=np.float32)
    for g in range(NG):
        for h in range(2):
            b = 2 * g + h
            s = b * B
            blk = Wm[s:s + B, s:s + B]
            wt_g[g, h * B:(h + 1) * B, h * B:(h + 1) * B] = blk.T
    # [g, r, c] -> [r, g*128 + c]
    return np.ascontiguousarray(wt_g.transpose(1, 0, 2).reshape(P, L))


def run(x, W, mask, trace=False, trace_cores=None, tmpdir=None):
    x = np.ascontiguousarray(np.asarray(x, dtype=np.float32))
    W = np.asarray(W, dtype=np.float32)
    mask = np.asarray(mask).astype(bool)
    assert x.shape == (ROWS, L) and W.shape == (L, L) and mask.shape == (L, L)

    if not _mask_is_block_diagonal(mask):
        # Safety net (never expected): mask is not block-diagonal, so the
        # effective weight is dense. Compute densely on host.
        blocks = np.tanh(np.where(mask, W, np.float32(0.0)).astype(np.float32))
        return (x @ blocks.T).astype(np.float32), None

    wt = _build_wt(W, mask)

    nc = _get_nc()
    in_maps = [
        {"x": x[c * R:(c + 1) * R, :], "wt": wt}
        for c in range(NCORES)
    ]
    res = run_bass_kernel_spmd(
        nc,
        in_maps,
        list(range(NCORES)),
        trace=trace,
        trace_cores=trace_cores,
        tmpdir=tmpdir,
    )
    out = np.concatenate([res.results[c]["out"] for c in range(NCORES)], axis=0)
    return out, res


def kernel(x, W, mask):
    out, _ = run(x, W, mask, trace=False)
    return out


# revision 35
# speedup vs baseline: 1.0378x; 1.0378x over previous
"""Trainium2 Bass kernel for nn_BlockDiagonal.

Math: out = x @ tanh(W * mask).T, where mask is block-diagonal with 64
blocks of 64x64. tanh(0) = 0, so the effective weight is block-diagonal:
out[:, 64j:64j+64] = x[:, 64j:64j+64] @ tanh(Wb_j).T for each block j —
64 independent small GEMMs, 1/64th of the dense FLOPs.

Strategy (8 NeuronCores, data parallel over rows; ~32MB of HBM traffic
per core makes this DMA/PE balanced — "ridge"):
- Each core owns 1024 rows of x / out.
- Host (numpy, data movement only): verify the mask is block-diagonal,
  gather the 64 diagonal 64x64 blocks of W (mask applied via np.where
  selection), pack pairs of blocks into [128,128] block-diagonal tiles,
  transposed, zeros off-block -> wt [128, 4096]. All arithmetic (tanh,
  matmul) runs on device; tanh maps the structural zeros to zero.
- Device per core, all fp32 (exact to ~1e-7):
  - weights: 4 [128,1024] quarter tiles DMA'd on the Scalar HWDGE ring
    (idle until stores), tanh'd in place on ACT.
  - x streams on the Sync HWDGE ring: chunk 0 in 512KB quarters (fast
    pipeline start), later chunks in 1MB halves. Separate rings for
    loads vs stores avoid FIFO head-of-line blocking.
  - per [128 rows, 1024 cols] batch: 8 PE transposes (identity built
    on-device) -> PSUM -> DVE copy to SBUF (xt), then 8 fp32 matmuls
    lhsT=xt[feat,rows], rhs=tanh-tile [feat_in,feat_out] -> PSUM ->
    ACT copy -> SBUF -> store. Matmul batches run one stage behind
    transpose batches so PE never waits on the DVE copy.
"""

import os
import sys

import numpy as np

for _p in ("/opt/trn_rl_repo", "/root/.axon_site/_ro/trn_rl_repo"):
    if os.path.isdir(_p) and _p not in sys.path:
        sys.path.append(_p)

from contextlib import ExitStack

# If BASS_TRACE is set in the environment, bass_utils' axon path imports
# antenv.axon_hooks, which this image lacks — stub it (hook=None means
# "run without tracing") so kernel() can never crash on it.
try:
    from antenv.axon_hooks import get_axon_ntff_profile_hook  # noqa: F401
except ImportError:
    import types

    import antenv

    _stub = types.ModuleType("antenv.axon_hooks")
    _stub._hook = None
    _stub.get_axon_ntff_profile_hook = lambda: _stub._hook

    def _set_hook(h):
        _stub._hook = h

    _stub.set_axon_ntff_profile_hook = _set_hook
    sys.modules["antenv.axon_hooks"] = _stub
    antenv.axon_hooks = _stub

import concourse.bacc as bacc
from concourse import mybir
from concourse.bass_utils import run_bass_kernel_spmd
from concourse.masks import make_identity
from concourse.tile import TileContext

ROWS = 8192
L = 4096
NB = 64           # number of diagonal blocks
B = 64            # block size
NCORES = 8
R = ROWS // NCORES  # rows per core = 1024
P = 128             # partition tile
NG = L // P         # 32 column groups (2 blocks each)
NCHUNK = R // P     # 8 row chunks per core

F32 = mybir.dt.float32

_NC_CACHE = {}


def build_nc():
    """Build the per-core Bass program (SPMD: same program, different data)."""
    nc = bacc.Bacc()
    x_in = nc.declare_dram_parameter("x", [R, L], F32, isOutput=False)
    wt_in = nc.declare_dram_parameter("wt", [P, L], F32, isOutput=False)
    out_ext = nc.declare_dram_parameter("out", [R, L], F32, isOutput=True)

    with TileContext(nc) as tc, ExitStack() as ctx:
        singles = ctx.enter_context(tc.tile_pool(name="singles", bufs=1))
        wpool = ctx.enter_context(tc.tile_pool(name="wpool", bufs=4))
        xpool = ctx.enter_context(tc.tile_pool(name="xpool", bufs=4))
        xtpool = ctx.enter_context(tc.tile_pool(name="xtpool", bufs=6))
        opool = ctx.enter_context(tc.tile_pool(name="opool", bufs=4))
        pt = ctx.enter_context(tc.tile_pool(name="pt", bufs=2, space="PSUM"))
        pm = ctx.enter_context(tc.tile_pool(name="pm", bufs=2, space="PSUM"))

        Q = 1024        # quarter-chunk columns = one batch of 8 column groups
        H = L // 2      # half-chunk columns (store granularity)

        ident = singles.tile([P, P], F32)
        make_identity(nc, ident)


        # x is loaded in quarter-chunks, one tile per (chunk, q) batch, so the
        # first transposes start as soon as 512KB has landed. Loads ride the
        # Sync HWDGE ring. Weights ride the Scalar ring in quarters too:
        # batch q's matmuls need exactly weight quarter q.
        def load_xq(ic, q):
            xq = xpool.tile([P, Q], F32, tag="xs", name=f"xq_{ic}_{q}", bufs=8)
            nc.sync.dma_start(
                out=xq, in_=x_in[ic * P:(ic + 1) * P, Q * q:Q * (q + 1)],
            )
            return xq

        first_q = {ic: [load_xq(ic, q) for q in range(4)] for ic in range(2)}

        wts = []
        for w in range(4):
            wq = wpool.tile([P, Q], F32, tag="wt", name=f"wt_{w}")
            # quarters 0-1 are needed almost immediately: HWDGE scalar ring.
            # quarters 2-3 are needed ~8us later: the idle GpSimd SWDGE queue
            # handles them without stealing early HWDGE bandwidth from x.
            eng = nc.scalar if w < 2 else nc.gpsimd
            eng.dma_start(out=wq, in_=wt_in[:, Q * w:Q * (w + 1)])
            wts.append(wq)
        for w in range(4):
            for s in range(2):
                nc.scalar.activation(
                    out=wts[w][:, 512 * s:512 * (s + 1)],
                    in_=wts[w][:, 512 * s:512 * (s + 1)],
                    func=mybir.ActivationFunctionType.Tanh,
                )

        def mm_batch(ic, q, xt, last, ohalves):
            po = pm.tile([P, Q], F32, tag="po", name=f"po_{ic}_{q}")
            for t in range(8):
                nc.tensor.matmul(
                    po[:, P * t:P * (t + 1)],
                    lhsT=xt[:, P * t:P * (t + 1)],
                    rhs=wts[q][:, P * t:P * (t + 1)],
                    start=True,
                    stop=True,
                )
            # PSUM->SBUF out copies on ACT (DVE stays dedicated to the xt
            # copies that feed PE); last chunk splits to DVE since no more
            # xt copies are coming.
            oh = ohalves[q // 2]
            dst = oh[:, Q * (q % 2):Q * (q % 2 + 1)]
            if last:
                # drain tail fast: halves copied on DVE and ACT in parallel
                nc.vector.tensor_copy(out=dst[:, 0:512], in_=po[:, 0:512])
                nc.scalar.copy(out=dst[:, 512:1024], in_=po[:, 512:1024])
            else:
                nc.scalar.copy(out=dst, in_=po)
            # stores ride the Scalar HWDGE ring; per half chunk, but
            # quarter-chunk on the last chunk to shorten the drain tail
            if last:
                nc.scalar.dma_start(
                    out=out_ext[ic * P:(ic + 1) * P, Q * q:Q * (q + 1)],
                    in_=dst,
                )
            elif q % 2 == 1:
                h = q // 2
                nc.scalar.dma_start(
                    out=out_ext[ic * P:(ic + 1) * P, h * H:(h + 1) * H],
                    in_=oh,
                )

        def load_xh(ic, h):
            xh = xpool.tile([P, H], F32, tag="xs2", name=f"xh_{ic}_{h}", bufs=6)
            nc.sync.dma_start(
                out=xh, in_=x_in[ic * P:(ic + 1) * P, h * H:(h + 1) * H],
            )
            return xh

        pending = None  # one-stage pipeline skew on PE
        for ic in range(NCHUNK):
            last = ic == NCHUNK - 1
            halves = None if ic < 2 else [load_xh(ic, h) for h in range(2)]
            ohalves = [
                opool.tile([P, H], F32, tag="outs", name=f"oh_{ic}_{h}")
                for h in range(2)
            ]
            for q in range(4):
                if ic < 2:
                    xq, off = first_q[ic][q], 0
                else:
                    xq, off = halves[q // 2], Q * (q % 2)
                tp = pt.tile([P, Q], F32, tag="tp", name=f"tp_{ic}_{q}")
                for t in range(8):
                    nc.tensor.transpose(
                        tp[:, P * t:P * (t + 1)],
                        xq[:, off + P * t:off + P * (t + 1)],
                        ident,
                    )
                xt = xtpool.tile([P, Q], F32, tag="xt", name=f"xt_{ic}_{q}")
                nc.vector.tensor_copy(out=xt, in_=tp)
                if pending is not None:
                    mm_batch(*pending)
                pending = (ic, q, xt, last, ohalves)
        mm_batch(*pending)

    nc.compile()
    return nc


def _get_nc():
    if "nc" not in _NC_CACHE:
        _NC_CACHE["nc"] = build_nc()
    return _NC_CACHE["nc"]


def _mask_is_block_diagonal(mask: np.ndarray) -> bool:
    off = mask.copy()
    for i in range(NB):
        s = i * B
        off[s:s + B, s:s + B] = False
    return not off.any()


def _build_wt(W: np.ndarray, mask: np.ndarray) -> np.ndarray:
    """Pack the 64 diagonal blocks, transposed, into [128, 4096]:
    wt[r, 128*g + c] = (W*mask).T within block pair g, zeros off-block."""
    Wm = np.where(mask, W, np.float32(0.0)).astype(np.float32)
    wt_g = np.zeros((NG, P, P), dtype=np.float32)
    for g in range(NG):
        for h in range(2):
            b = 2 * g + h
            s = b * B
            blk = Wm[s:s + B, s:s + B]
            wt_g[g, h * B:(h + 1) * B, h * B:(h + 1) * B] = blk.T
    # [g, r, c] -> [r, g*128 + c]
    return np.ascontiguousarray(wt_g.transpose(1, 0, 2).reshape(P, L))


def run(x, W, mask, trace=False, trace_cores=None, tmpdir=None):
    x = np.ascontiguousarray(np.asarray(x, dtype=np.float32))
    W = np.asarray(W, dtype=np.float32)
    mask = np.asarray(mask).astype(bool)
    assert x.shape == (ROWS, L) and W.shape == (L, L) and mask.shape == (L, L)

    if not _mask_is_block_diagonal(mask):
        # Safety net (never expected): mask is not block-diagonal, so the
        # effective weight is dense. Compute densely on host.
        blocks = np.tanh(np.where(mask, W, np.float32(0.0)).astype(np.float32))
        return (x @ blocks.T).astype(np.float32), None

    wt = _build_wt(W, mask)

    nc = _get_nc()
    in_maps = [
        {"x": x[c * R:(c + 1) * R, :], "wt": wt}
        for c in range(NCORES)
    ]
    res = run_bass_kernel_spmd(
        nc,
        in_maps,
        list(range(NCORES)),
        trace=trace,
        trace_cores=trace_cores,
        tmpdir=tmpdir,
    )
    out = np.concatenate([res.results[c]["out"] for c in range(NCORES)], axis=0)
    return out, res


def kernel(x, W, mask):
    out, _ = run(x, W, mask, trace=False)
    return out


# revision 36
# speedup vs baseline: 1.0404x; 1.0025x over previous
"""Trainium2 Bass kernel for nn_BlockDiagonal.

Math: out = x @ tanh(W * mask).T, where mask is block-diagonal with 64
blocks of 64x64. tanh(0) = 0, so the effective weight is block-diagonal:
out[:, 64j:64j+64] = x[:, 64j:64j+64] @ tanh(Wb_j).T for each block j —
64 independent small GEMMs, 1/64th of the dense FLOPs.

Strategy (8 NeuronCores, data parallel over rows; ~32MB of HBM traffic
per core makes this DMA/PE balanced — "ridge"):
- Each core owns 1024 rows of x / out.
- Host (numpy, data movement only): verify the mask is block-diagonal,
  gather the 64 diagonal 64x64 blocks of W (mask applied via np.where
  selection), pack pairs of blocks into [128,128] block-diagonal tiles,
  transposed, zeros off-block -> wt [128, 4096]. All arithmetic (tanh,
  matmul) runs on device; tanh maps the structural zeros to zero.
- Device per core, all fp32 (exact to ~1e-7):
  - weights: 4 [128,1024] quarter tiles DMA'd on the Scalar HWDGE ring
    (idle until stores), tanh'd in place on ACT.
  - x streams on the Sync HWDGE ring: chunk 0 in 512KB quarters (fast
    pipeline start), later chunks in 1MB halves. Separate rings for
    loads vs stores avoid FIFO head-of-line blocking.
  - per [128 rows, 1024 cols] batch: 8 PE transposes (identity built
    on-device) -> PSUM -> DVE copy to SBUF (xt), then 8 fp32 matmuls
    lhsT=xt[feat,rows], rhs=tanh-tile [feat_in,feat_out] -> PSUM ->
    ACT copy -> SBUF -> store. Matmul batches run one stage behind
    transpose batches so PE never waits on the DVE copy.
"""

import os
import sys

import numpy as np

for _p in ("/opt/trn_rl_repo", "/root/.axon_site/_ro/trn_rl_repo"):
    if os.path.isdir(_p) and _p not in sys.path:
        sys.path.append(_p)

from contextlib import ExitStack

# If BASS_TRACE is set in the environment, bass_utils' axon path imports
# antenv.axon_hooks, which this image lacks — stub it (hook=None means
# "run without tracing") so kernel() can never crash on it.
try:
    from antenv.axon_hooks import get_axon_ntff_profile_hook  # noqa: F401
except ImportError:
    import types

    import antenv

    _stub = types.ModuleType("antenv.axon_hooks")
    _stub._hook = None
    _stub.get_axon_ntff_profile_hook = lambda: _stub._hook

    def _set_hook(h):
        _stub._hook = h

    _stub.set_axon_ntff_profile_hook = _set_hook
    sys.modules["antenv.axon_hooks"] = _stub
    antenv.axon_hooks = _stub

import concourse.bacc as bacc
from concourse import mybir
from concourse.bass_utils import run_bass_kernel_spmd
from concourse.masks import make_identity
from concourse.tile import TileContext

ROWS = 8192
L = 4096
NB = 64           # number of diagonal blocks
B = 64            # block size
NCORES = 8
R = ROWS // NCORES  # rows per core = 1024
P = 128             # partition tile
NG = L // P         # 32 column groups (2 blocks each)
NCHUNK = R // P     # 8 row chunks per core

F32 = mybir.dt.float32

_NC_CACHE = {}


def build_nc():
    """Build the per-core Bass program (SPMD: same program, different data)."""
    nc = bacc.Bacc()
    x_in = nc.declare_dram_parameter("x", [R, L], F32, isOutput=False)
    wt_in = nc.declare_dram_parameter("wt", [P, L], F32, isOutput=False)
    out_ext = nc.declare_dram_parameter("out", [R, L], F32, isOutput=True)

    with TileContext(nc) as tc, ExitStack() as ctx:
        singles = ctx.enter_context(tc.tile_pool(name="singles", bufs=1))
        wpool = ctx.enter_context(tc.tile_pool(name="wpool", bufs=4))
        xpool = ctx.enter_context(tc.tile_pool(name="xpool", bufs=4))
        xtpool = ctx.enter_context(tc.tile_pool(name="xtpool", bufs=6))
        opool = ctx.enter_context(tc.tile_pool(name="opool", bufs=4))
        pt = ctx.enter_context(tc.tile_pool(name="pt", bufs=2, space="PSUM"))
        pm = ctx.enter_context(tc.tile_pool(name="pm", bufs=2, space="PSUM"))

        Q = 1024        # quarter-chunk columns = one batch of 8 column groups
        H = L // 2      # half-chunk columns (store granularity)

        ident = singles.tile([P, P], F32)
        make_identity(nc, ident)


        # x is loaded in quarter-chunks, one tile per (chunk, q) batch, so the
        # first transposes start as soon as 512KB has landed. Loads ride the
        # Sync HWDGE ring. Weights ride the Scalar ring in quarters too:
        # batch q's matmuls need exactly weight quarter q.
        def load_xq(ic, q):
            xq = xpool.tile([P, Q], F32, tag="xs", name=f"xq_{ic}_{q}", bufs=8)
            nc.sync.dma_start(
                out=xq, in_=x_in[ic * P:(ic + 1) * P, Q * q:Q * (q + 1)],
            )
            return xq

        first_q = {ic: [load_xq(ic, q) for q in range(4)] for ic in range(2)}

        wts = []
        for w in range(4):
            wq = wpool.tile([P, Q], F32, tag="wt", name=f"wt_{w}")
            # quarters 0-1 are needed almost immediately: HWDGE scalar ring.
            # quarters 2-3 are needed ~8us later: the idle GpSimd SWDGE queue
            # handles them without stealing early HWDGE bandwidth from x.
            eng = nc.scalar if w < 2 else nc.gpsimd
            eng.dma_start(out=wq, in_=wt_in[:, Q * w:Q * (w + 1)])
            wts.append(wq)
        for w in range(4):
            for s in range(2):
                nc.scalar.activation(
                    out=wts[w][:, 512 * s:512 * (s + 1)],
                    in_=wts[w][:, 512 * s:512 * (s + 1)],
                    func=mybir.ActivationFunctionType.Tanh,
                )

        def mm_batch(ic, q, xt, last, ohalves):
            po = pm.tile([P, Q], F32, tag="po", name=f"po_{ic}_{q}")
            for t in range(8):
                nc.tensor.matmul(
                    po[:, P * t:P * (t + 1)],
                    lhsT=xt[:, P * t:P * (t + 1)],
                    rhs=wts[q][:, P * t:P * (t + 1)],
                    start=True,
                    stop=True,
                )
            # PSUM->SBUF out copies on ACT (DVE stays dedicated to the xt
            # copies that feed PE); last chunk splits to DVE since no more
            # xt copies are coming.
            oh = ohalves[q // 2]
            dst = oh[:, Q * (q % 2):Q * (q % 2 + 1)]
            if last:
                # drain tail fast: halves copied on DVE and ACT in parallel
                nc.vector.tensor_copy(out=dst[:, 0:512], in_=po[:, 0:512])
                nc.scalar.copy(out=dst[:, 512:1024], in_=po[:, 512:1024])
            else:
                nc.scalar.copy(out=dst, in_=po)
            # stores ride the Scalar HWDGE ring; per half chunk. The final
            # two chunks store on the Sync ring instead (its loads are done
            # by then), so store issues don't serialize behind ACT copies —
            # and the last chunk stores per quarter to shorten the tail.
            seng = nc.sync if ic >= NCHUNK - 2 else nc.scalar
            if last:
                seng.dma_start(
                    out=out_ext[ic * P:(ic + 1) * P, Q * q:Q * (q + 1)],
                    in_=dst,
                )
            elif q % 2 == 1:
                h = q // 2
                seng.dma_start(
                    out=out_ext[ic * P:(ic + 1) * P, h * H:(h + 1) * H],
                    in_=oh,
                )

        def load_xh(ic, h):
            xh = xpool.tile([P, H], F32, tag="xs2", name=f"xh_{ic}_{h}", bufs=6)
            nc.sync.dma_start(
                out=xh, in_=x_in[ic * P:(ic + 1) * P, h * H:(h + 1) * H],
            )
            return xh

        pending = None  # one-stage pipeline skew on PE
        for ic in range(NCHUNK):
            last = ic == NCHUNK - 1
            halves = None if ic < 2 else [load_xh(ic, h) for h in range(2)]
            ohalves = [
                opool.tile([P, H], F32, tag="outs", name=f"oh_{ic}_{h}")
                for h in range(2)
            ]
            for q in range(4):
                if ic < 2:
                    xq, off = first_q[ic][q], 0
                else:
                    xq, off = halves[q // 2], Q * (q % 2)
                tp = pt.tile([P, Q], F32, tag="tp", name=f"tp_{ic}_{q}")
                for t in range(8):
                    nc.tensor.transpose(
                        tp[:, P * t:P * (t + 1)],
                        xq[:, off + P * t:off + P * (t + 1)],
                        ident,
                    )
                xt = xtpool.tile([P, Q], F32, tag="xt", name=f"xt_{ic}_{q}")
                nc.vector.tensor_copy(out=xt, in_=tp)
                if pending is not None:
                    mm_batch(*pending)
                pending = (ic, q, xt, last, ohalves)
        mm_batch(*pending)

    nc.compile()
    return nc


def _get_nc():
    if "nc" not in _NC_CACHE:
        _NC_CACHE["nc"] = build_nc()
    return _NC_CACHE["nc"]


def _mask_is_block_diagonal(mask: np.ndarray) -> bool:
    off = mask.copy()
    for i in range(NB):
        s = i * B
        off[s:s + B, s:s + B] = False
    return not off.any()


def _build_wt(W: np.ndarray, mask: np.ndarray) -> np.ndarray:
    """Pack the 64 diagonal blocks, transposed, into [128, 4096]:
    wt[r, 128*g + c] = (W*mask).T within block pair g, zeros off-block."""
    Wm = np.where(mask, W, np.float32(0.0)).astype(np.float32)
    wt_g = np.zeros((NG, P, P), dtype=np.float32)
    for g in range(NG):
        for h in range(2):
            b = 2 * g + h
            s = b * B
            blk = Wm[s:s + B, s:s + B]
            wt_g[g, h * B:(h + 1) * B, h * B:(h + 1) * B] = blk.T
    # [g, r, c] -> [r, g*128 + c]
    return np.ascontiguousarray(wt_g.transpose(1, 0, 2).reshape(P, L))


def run(x, W, mask, trace=False, trace_cores=None, tmpdir=None):
    x = np.ascontiguousarray(np.asarray(x, dtype=np.float32))
    W = np.asarray(W, dtype=np.float32)
    mask = np.asarray(mask).astype(bool)
    assert x.shape == (ROWS, L) and W.shape == (L, L) and mask.shape == (L, L)

    if not _mask_is_block_diagonal(mask):
        # Safety net (never expected): mask is not block-diagonal, so the
        # effective weight is dense. Compute densely on host.
        blocks = np.tanh(np.where(mask, W, np.float32(0.0)).astype(np.float32))
        return (x @ blocks.T).astype(np.float32), None

    wt = _build_wt(W, mask)

    nc = _get_nc()
    in_maps = [
        {"x": x[c * R:(c + 1) * R, :], "wt": wt}
        for c in range(NCORES)
    ]
    res = run_bass_kernel_spmd(
        nc,
        in_maps,
        list(range(NCORES)),
        trace=trace,
        trace_cores=trace_cores,
        tmpdir=tmpdir,
    )
    out = np.concatenate([res.results[c]["out"] for c in range(NCORES)], axis=0)
    return out, res


def kernel(x, W, mask):
    out, _ = run(x, W, mask, trace=False)
    return out
